# revision 4
# baseline (speedup 1.0000x reference)
"""Trainium2 Bass kernel for nn_DSCAMSFF (1x1 conv + per-group CBAM gating).

Only x4 is live in the reference model (cov1-3 / the attention path are dead
code that returns its first argument). Effective computation per batch b:

  a  = conv1x1(x4[b]) : [512, 256]          (w [512,2048], pixels flattened)
  x  = concat([a]*4)  : [2048, 256] in 8 groups of 256 channels
  per group g (channels of group g are a[(g%2)*256 : (g%2+1)*256]):
    avg_g = mean_px(a_g)                       [256]
    h_g   = relu(fc1_w[g] @ avg_g + fc1_b[g])  [64]
    ca_g  = sigmoid(fc2_w[g] @ h_g + fc2_b[g]) [256]
    sa_g  = sigmoid((ca_g*sa_w[g]) . a_g + sa_b[g])   [256 px]
    z_g   = sigmoid(a_g * ca_g[:,None] * sa_g[None,:])
    mean_g = mean(z_g)
    mask  = where(z_g > mean_g, 1, z_g)
    out_g = a_g * (mask + 1)

Sharding: pure data-parallel over batch (8 cores x 1 batch element),
parameters replicated. All host-side layouts are pre-arranged so every DMA is
a contiguous partition-major copy.
"""

import numpy as np

N_CORES = 8
P = 128
PX = 256            # 16*16 pixels
KT = 16             # 2048 / 128 K tiles
MT = 4              # 512 / 128 conv out tiles

# column offsets inside the packed small-parameter tensor
_W1_OFF = 0          # [p, kt, mm]   2*2*256  = 1024
_W2_OFF = 1024       # [p, pr, s, m] 2*2*2*128 = 1024
_B_OFF = 2048        # [m] 4
_B1_OFF = 2052       # [p, mt] 4
_B2_OFF = 2056       # [p, s, i] 16
_SAW_OFF = 2072      # [p, s, i] 16
_SAB_OFF = 2088      # [g] 8
_NSMALL = 2096

_CACHE = {}


def _build_program():
    import concourse.mybir as mybir
    import concourse.tile as tile
    from concourse import bacc, bass_isa

    fp32 = mybir.dt.float32
    Act = mybir.ActivationFunctionType
    Alu = mybir.AluOpType

    nc = bacc.Bacc("TRN2", target_bir_lowering=False, debug=False)

    x_d = nc.dram_tensor("x", [P, KT, PX], fp32, kind="ExternalInput").ap()
    w_d = nc.dram_tensor("w", [MT, P, KT, P], fp32, kind="ExternalInput").ap()
    sm_d = nc.dram_tensor("smalls", [P, _NSMALL], fp32, kind="ExternalInput").ap()
    out_d = nc.dram_tensor("out", [16, P, PX], fp32, kind="ExternalOutput").ap()

    with tile.TileContext(nc) as tc:
        with (
            tc.tile_pool(name="singles", bufs=1) as singles,
            tc.tile_pool(name="work", bufs=3) as work,
            tc.tile_pool(name="zpool", bufs=9) as zpool,
            tc.tile_pool(name="psA", bufs=2, space="PSUM") as psA,
            tc.tile_pool(name="psB", bufs=2, space="PSUM") as psB,
        ):
            # ---- input DMAs (issue order ~= service order on the queues) ----
            xk = []
            for j in range(4):
                t = singles.tile([P, 4, PX], fp32, tag=f"x{j}", name=f"x{j}")
                nc.sync.dma_start(out=t, in_=x_d[:, 4 * j:4 * j + 4, :])
                xk.append(t)

            wt = [None] * MT
            for m in (0, 1):
                wt[m] = singles.tile([P, KT, P], fp32, tag=f"w{m}", name=f"w{m}")
                nc.sync.dma_start(out=wt[m], in_=w_d[m])

            smalls = singles.tile([P, _NSMALL], fp32, tag="smalls")
            nc.sync.dma_start(out=smalls, in_=sm_d)

            for m in (2, 3):
                wt[m] = singles.tile([P, KT, P], fp32, tag=f"w{m}", name=f"w{m}")
                nc.sync.dma_start(out=wt[m], in_=w_d[m])

            # parameter views
            w1v = smalls[:, _W1_OFF:_W1_OFF + 1024].rearrange(
                "P (p k m) -> P p k m", p=2, k=2)
            w2v = smalls[:, _W2_OFF:_W2_OFF + 1024].rearrange(
                "P (p r s m) -> P p r s m", p=2, r=2, s=2)
            bv = smalls[:, _B_OFF:_B_OFF + 4]
            b1v = smalls[:, _B1_OFF:_B1_OFF + 4].rearrange("P (p t) -> P p t", p=2)
            b2v = smalls[:, _B2_OFF:_B2_OFF + 16].rearrange(
                "P (p s i) -> P p s i", p=2, s=2)
            sawv = smalls[:, _SAW_OFF:_SAW_OFF + 16].rearrange(
                "P (p s i) -> P p s i", p=2, s=2)
            sabv = smalls[:, _SAB_OFF:_SAB_OFF + 8]

            a_sb = [None] * MT
            asum = [None, None]   # per parity [128, 2]
            h_sb = [None, None]
            h_m = [None, None]
            ca = [None, None]
            weff = [None, None]

            def conv_m(m):
                ps = psA.tile([P, PX], fp32, tag="conv")
                for kt in range(KT):
                    nc.tensor.matmul(
                        ps, lhsT=wt[m][:, kt, :], rhs=xk[kt // 4][:, kt % 4, :],
                        start=(kt == 0), stop=(kt == KT - 1))
                a_sb[m] = singles.tile([P, PX], fp32, tag=f"a{m}", name=f"a{m}")
                p = m // 2
                if asum[p] is None:
                    asum[p] = singles.tile([P, 2], fp32, tag=f"asum{p}", name=f"asum{p}")
                nc.scalar.activation(
                    out=a_sb[m], in_=ps, func=Act.Identity,
                    bias=bv[:, m:m + 1], scale=1.0,
                    accum_out=asum[p][:, m % 2:m % 2 + 1])

            def fc_chain(p):
                # fc1: h = relu(W1/256 @ asum + b1), 4 groups of 64 stacked
                h_sb[p] = singles.tile([P, 2], fp32, tag=f"h{p}", name=f"h{p}")
                for mt in (0, 1):
                    hp = psB.tile([P, 1], fp32, tag="tiny")
                    for kt in (0, 1):
                        nc.tensor.matmul(
                            hp, lhsT=w1v[:, p, kt, mt * P:(mt + 1) * P],
                            rhs=asum[p][:, kt:kt + 1],
                            start=(kt == 0), stop=(kt == 1))
                    nc.scalar.activation(
                        out=h_sb[p][:, mt:mt + 1], in_=hp, func=Act.Relu,
                        bias=b1v[:, p, mt:mt + 1], scale=1.0)
                # masked h per group (zero the other 64-row half)
                h_m[p] = singles.tile([P, 4], fp32, tag=f"hm{p}", name=f"hm{p}")
                nc.gpsimd.memset(h_m[p], 0.0)
                for i in range(4):
                    lo = 64 * (i % 2)
                    nc.gpsimd.tensor_copy(
                        out=h_m[p][lo:lo + 64, i:i + 1],
                        in_=h_sb[p][lo:lo + 64, i // 2:i // 2 + 1])
                # fc2: ca = sigmoid(W2 @ h + b2)
                ca[p] = singles.tile([P, 2, 4], fp32, tag=f"ca{p}", name=f"ca{p}")
                for s in (0, 1):
                    for i in range(4):
                        cp = psB.tile([P, 1], fp32, tag="tiny")
                        nc.tensor.matmul(
                            cp, lhsT=w2v[:, p, i // 2, s, :],
                            rhs=h_m[p][:, i:i + 1], start=True, stop=True)
                        nc.scalar.activation(
                            out=ca[p][:, s, i:i + 1], in_=cp, func=Act.Sigmoid,
                            bias=b2v[:, p, s, i:i + 1], scale=1.0)
                weff[p] = singles.tile([P, 2, 4], fp32, tag=f"we{p}", name=f"we{p}")
                nc.vector.tensor_mul(weff[p], ca[p], sawv[:, p])

            def gate_p(p):
                # spatial attention + z = sigmoid(a * ca * sa) per group
                zs = []
                zsum = singles.tile([P, 4, 2], fp32, tag=f"zs{p}")
                for i in range(4):
                    g = p + 2 * i
                    sps = psA.tile([P, PX], fp32, tag="sa")
                    for s in (0, 1):
                        wr = work.tile([P, P], fp32, tag="wrep")
                        nc.vector.tensor_copy(
                            out=wr, in_=weff[p][:, s, i:i + 1].to_broadcast((P, P)))
                        nc.tensor.matmul(
                            sps, lhsT=wr, rhs=a_sb[2 * p + s],
                            start=(s == 0), stop=(s == 1))
                    sarep = work.tile([P, PX], fp32, tag="sarep")
                    nc.scalar.activation(
                        out=sarep, in_=sps, func=Act.Sigmoid,
                        bias=sabv[:, g:g + 1], scale=1.0)
                    for s in (0, 1):
                        t = work.tile([P, PX], fp32, tag="t")
                        nc.vector.tensor_mul(t, a_sb[2 * p + s], sarep)
                        z = zpool.tile([P, PX], fp32, tag="z")
                        nc.scalar.activation(
                            out=z, in_=t, func=Act.Sigmoid,
                            scale=ca[p][:, s, i:i + 1],
                            accum_out=zsum[:, i, s:s + 1])
                        zs.append((i, s, z))
                # group means (negated, scaled) replicated on all partitions
                zr = singles.tile([P, 4, 2], fp32, tag=f"zr{p}")
                nc.gpsimd.partition_all_reduce(
                    zr, zsum, channels=P, reduce_op=bass_isa.ReduceOp.add)
                nm = singles.tile([P, 4], fp32, tag=f"nm{p}")
                nc.vector.tensor_reduce(nm, zr, axis=mybir.AxisListType.X, op=Alu.add)
                nc.vector.tensor_scalar_mul(nm, nm, -1.0 / 65536.0)
                # mask + output
                for k, (i, s, z) in enumerate(zs):
                    g = p + 2 * i
                    sg = work.tile([P, PX], fp32, tag="sg")
                    nc.scalar.activation(
                        out=sg, in_=z, func=Act.Sign,
                        bias=nm[:, i:i + 1], scale=1.0)
                    m1 = work.tile([P, PX], fp32, tag="m1")
                    nc.vector.tensor_tensor(out=m1, in0=z, in1=sg, op=Alu.max)
                    nc.scalar.add(m1, m1, 1.0)
                    ot = work.tile([P, PX], fp32, tag="ot")
                    nc.vector.tensor_mul(ot, a_sb[2 * p + s], m1)
                    nc.sync.dma_start(out=out_d[g * 2 + s], in_=ot)

            conv_m(0)
            conv_m(1)
            fc_chain(0)
            gate_p(0)
            conv_m(2)
            conv_m(3)
            fc_chain(1)
            gate_p(1)

    nc.finalize()
    return nc


def _prep_core_inputs(x4b, w, smalls):
    # x: [128, 16, 256] partition-major K tiles
    x = np.ascontiguousarray(
        x4b.reshape(KT, P, PX).transpose(1, 0, 2))
    return {"x": x, "w": w, "smalls": smalls}


def _prep_params(cov4_w, cov4_b, fc1_w, fc1_b, fc2_w, fc2_b, sa_w, sa_b):
    f32 = np.float32
    w2d = np.asarray(cov4_w, f32).reshape(512, 2048)          # [out, in]
    # w dram: [m, part, kt, mc] = w[m*128+mc, kt*128+part]
    wr = w2d.reshape(MT, P, KT, P)                            # [m, mc, kt, part]
    w_arr = np.ascontiguousarray(wr.transpose(0, 3, 2, 1))    # [m, part, kt, mc]

    smalls = np.zeros((P, _NSMALL), f32)
    fc1_w = np.asarray(fc1_w, f32)
    fc1_b = np.asarray(fc1_b, f32)
    fc2_w = np.asarray(fc2_w, f32)
    fc2_b = np.asarray(fc2_b, f32)
    sa_w = np.asarray(sa_w, f32)
    sa_b = np.asarray(sa_b, f32)

    w1 = np.zeros((P, 2, 2, 256), f32)
    b1 = np.zeros((P, 2, 2), f32)
    w2 = np.zeros((P, 2, 2, 2, P), f32)
    b2 = np.zeros((P, 2, 2, 4), f32)
    saw = np.zeros((P, 2, 2, 4), f32)
    for p in range(2):
        # stacked fc1 for groups p, p+2, p+4, p+6 ; scaled by 1/256 (pixel mean)
        W1s = np.concatenate([fc1_w[p + 2 * i] for i in range(4)], axis=0) / 256.0
        b1s = np.concatenate([fc1_b[p + 2 * i] for i in range(4)], axis=0)
        for kt in range(2):
            w1[:, p, kt, :] = W1s[:, kt * P:(kt + 1) * P].T      # [part(k), mm]
        b1[:, p, 0] = b1s[:P]
        b1[:, p, 1] = b1s[P:]
        for pr in range(2):
            for half in range(2):
                i = 2 * pr + half
                g = p + 2 * i
                for s in range(2):
                    # lhsT rows [64*half, 64*half+64) = fc2_w[g][s*128:(s+1)*128, :].T
                    w2[64 * half:64 * half + 64, p, pr, s, :] = \
                        fc2_w[g][s * P:(s + 1) * P, :].T
        for i in range(4):
            g = p + 2 * i
            for s in range(2):
                b2[:, p, s, i] = fc2_b[g, s * P:(s + 1) * P]
                saw[:, p, s, i] = sa_w[g, s * P:(s + 1) * P]

    smalls[:, _W1_OFF:_W1_OFF + 1024] = w1.reshape(P, 1024)
    smalls[:, _W2_OFF:_W2_OFF + 1024] = w2.reshape(P, 1024)
    smalls[:, _B_OFF:_B_OFF + 4] = np.asarray(cov4_b, f32).reshape(4, P).T
    smalls[:, _B1_OFF:_B1_OFF + 4] = b1.reshape(P, 4)
    smalls[:, _B2_OFF:_B2_OFF + 16] = b2.reshape(P, 16)
    smalls[:, _SAW_OFF:_SAW_OFF + 16] = saw.reshape(P, 16)
    smalls[:, _SAB_OFF:_SAB_OFF + 8] = np.broadcast_to(sa_b, (P, 8))
    return w_arr, smalls


def kernel(**inputs):
    from concourse.bass_utils import run_bass_kernel_spmd

    if "nc" not in _CACHE:
        _CACHE["nc"] = _build_program()
    nc = _CACHE["nc"]

    x4 = np.asarray(inputs["x4"], np.float32)
    B = x4.shape[0]
    w_arr, smalls = _prep_params(
        inputs["cov4_w"], inputs["cov4_b"],
        inputs["gce_fc1_w"], inputs["gce_fc1_b"],
        inputs["gce_fc2_w"], inputs["gce_fc2_b"],
        inputs["gce_sa_w"], inputs["gce_sa_b"])

    in_maps = [
        _prep_core_inputs(x4[b].reshape(2048, PX), w_arr, smalls)
        for b in range(B)
    ]
    res = run_bass_kernel_spmd(nc, in_maps, list(range(N_CORES)))
    _CACHE["last_results"] = res

    out = np.empty((B, 2048, 16, 16), np.float32)
    for b in range(B):
        out[b] = res.results[b]["out"].reshape(2048, 16, 16)
    return out


# revision 5
# speedup vs baseline: 1.2927x; 1.2927x over previous
"""Trainium2 Bass kernel for nn_DSCAMSFF (1x1 conv + per-group CBAM gating).

Only x4 is live in the reference model (cov1-3 / the attention path are dead
code that returns its first argument). Effective computation per batch b:

  a  = conv1x1(x4[b]) : [512, 256]          (w [512,2048], pixels flattened)
  x  = concat([a]*4)  : [2048, 256] in 8 groups of 256 channels
  per group g (channels of group g are a[(g%2)*256 : (g%2+1)*256]):
    avg_g = mean_px(a_g)                       [256]
    h_g   = relu(fc1_w[g] @ avg_g + fc1_b[g])  [64]
    ca_g  = sigmoid(fc2_w[g] @ h_g + fc2_b[g]) [256]
    sa_g  = sigmoid((ca_g*sa_w[g]) . a_g + sa_b[g])   [256 px]
    z_g   = sigmoid(a_g * ca_g[:,None] * sa_g[None,:])
    mean_g = mean(z_g)
    mask  = where(z_g > mean_g, 1, z_g)
    out_g = a_g * (mask + 1)

Sharding: pure data-parallel over batch (8 cores x 1 batch element),
parameters replicated. All matmuls run in fp16 (fp32 PSUM accumulate); the
gating arithmetic and the final multiply stay fp32. All host-side layouts are
pre-arranged so every DMA is a contiguous partition-major copy.
"""

import numpy as np

N_CORES = 8
P = 128
PX = 256            # 16*16 pixels
KT = 16             # 2048 / 128 K tiles
MT = 4              # 512 / 128 conv out tiles

# fp16 packed weights: w1 [p, kt, mm] 1024 cols + w2 [p, pr, s, m] 1024 cols
_NSM16 = 2048
# fp32 packed params: b [m] 4 | b1 [p, mt] 4 | b2 [p,s,i] 16 | saw [p,s,i] 16
# | sab [g] 8
_B_OFF = 0
_B1_OFF = 4
_B2_OFF = 8
_SAW_OFF = 24
_SAB_OFF = 40
_NSM32 = 48

_CACHE = {}


def _build_program():
    import concourse.mybir as mybir
    import concourse.tile as tile
    from concourse import bacc, bass_isa

    fp32 = mybir.dt.float32
    fp16 = mybir.dt.float16
    Act = mybir.ActivationFunctionType
    Alu = mybir.AluOpType

    nc = bacc.Bacc("TRN2", target_bir_lowering=False, debug=False)

    x_d = nc.dram_tensor("x", [P, KT, PX], fp16, kind="ExternalInput").ap()
    w_d = nc.dram_tensor("w", [MT, P, KT, P], fp16, kind="ExternalInput").ap()
    s16_d = nc.dram_tensor("s16", [P, _NSM16], fp16, kind="ExternalInput").ap()
    s32_d = nc.dram_tensor("s32", [P, _NSM32], fp32, kind="ExternalInput").ap()
    out_d = nc.dram_tensor("out", [16, P, PX], fp32, kind="ExternalOutput").ap()

    with tile.TileContext(nc) as tc:
        with (
            tc.tile_pool(name="singles", bufs=1) as singles,
            tc.tile_pool(name="work", bufs=3) as work,
            tc.tile_pool(name="zpool", bufs=5) as zpool,
            tc.tile_pool(name="psA", bufs=2, space="PSUM") as psA,
            tc.tile_pool(name="psB", bufs=2, space="PSUM") as psB,
        ):
            # ---- input DMAs, split across two issue queues ----
            xt = singles.tile([P, KT, PX], fp16, tag="xt")
            nc.sync.dma_start(out=xt, in_=x_d)

            wt = [None] * MT
            for m in (0, 1):
                wt[m] = singles.tile([P, KT, P], fp16, tag=f"w{m}", name=f"w{m}")
                nc.sync.dma_start(out=wt[m], in_=w_d[m])

            s16 = singles.tile([P, _NSM16], fp16, tag="s16")
            nc.gpsimd.dma_start(out=s16, in_=s16_d)
            s32 = singles.tile([P, _NSM32], fp32, tag="s32")
            nc.gpsimd.dma_start(out=s32, in_=s32_d)

            for m in (2, 3):
                wt[m] = singles.tile([P, KT, P], fp16, tag=f"w{m}", name=f"w{m}")
                nc.gpsimd.dma_start(out=wt[m], in_=w_d[m])

            # parameter views
            w1v = s16[:, 0:1024].rearrange("P (p k m) -> P p k m", p=2, k=2)
            w2v = s16[:, 1024:2048].rearrange("P (p r s m) -> P p r s m",
                                              p=2, r=2, s=2)
            bv = s32[:, _B_OFF:_B_OFF + 4]
            b1v = s32[:, _B1_OFF:_B1_OFF + 4].rearrange("P (p t) -> P p t", p=2)
            b2v = s32[:, _B2_OFF:_B2_OFF + 16].rearrange(
                "P (p s i) -> P p s i", p=2, s=2)
            sawv = s32[:, _SAW_OFF:_SAW_OFF + 16].rearrange(
                "P (p s i) -> P p s i", p=2, s=2)
            sabv = s32[:, _SAB_OFF:_SAB_OFF + 8]

            a_sb = [None, None]     # fp32 conv out, [128, 2, 256] per parity
            a16 = [None, None]      # fp16 copy for the spatial matmul
            asum = [None, None]
            asum16 = [None, None]
            h_sb = [None, None]
            h_m = [None, None]
            ca = [None, None]
            weff16 = [None, None]

            def conv_m(m):
                p, s = m // 2, m % 2
                if a_sb[p] is None:
                    a_sb[p] = singles.tile([P, 2, PX], fp32, tag=f"a{p}",
                                           name=f"a{p}")
                    asum[p] = singles.tile([P, 2], fp32, tag=f"as{p}",
                                           name=f"as{p}")
                ps = psA.tile([P, PX], fp32, tag="conv")
                for kt in range(KT):
                    nc.tensor.matmul(
                        ps, lhsT=wt[m][:, kt, :], rhs=xt[:, kt, :],
                        start=(kt == 0), stop=(kt == KT - 1))
                nc.scalar.activation(
                    out=a_sb[p][:, s, :], in_=ps, func=Act.Identity,
                    bias=bv[:, m:m + 1], scale=1.0,
                    accum_out=asum[p][:, s:s + 1])

            def fc_chain(p):
                # fp16 casts of conv results for the fp16 matmuls
                a16[p] = singles.tile([P, 2, PX], fp16, tag=f"a16_{p}",
                                      name=f"a16_{p}")
                nc.vector.tensor_copy(out=a16[p], in_=a_sb[p])
                asum16[p] = singles.tile([P, 2], fp16, tag=f"as16_{p}",
                                         name=f"as16_{p}")
                # fold the 1/256 pixel-mean into the cast (not into fp16
                # weights: 0.02/256 would be subnormal in fp16)
                nc.scalar.mul(out=asum16[p], in_=asum[p], mul=1.0 / 256.0)
                # fc1: h = relu(W1 @ avg + b1), 4 groups of 64 stacked
                h_sb[p] = singles.tile([P, 2], fp16, tag=f"h{p}", name=f"h{p}")
                for mt in (0, 1):
                    hp = psB.tile([P, 1], fp32, tag="tiny")
                    for kt in (0, 1):
                        nc.tensor.matmul(
                            hp, lhsT=w1v[:, p, kt, mt * P:(mt + 1) * P],
                            rhs=asum16[p][:, kt:kt + 1],
                            start=(kt == 0), stop=(kt == 1))
                    nc.scalar.activation(
                        out=h_sb[p][:, mt:mt + 1], in_=hp, func=Act.Relu,
                        bias=b1v[:, p, mt:mt + 1], scale=1.0)
                # masked h per group (zero the other 64-row half)
                h_m[p] = singles.tile([P, 4], fp16, tag=f"hm{p}", name=f"hm{p}")
                nc.gpsimd.memset(h_m[p], 0.0)
                for i in range(4):
                    lo = 64 * (i % 2)
                    nc.gpsimd.tensor_copy(
                        out=h_m[p][lo:lo + 64, i:i + 1],
                        in_=h_sb[p][lo:lo + 64, i // 2:i // 2 + 1])
                # fc2: ca = sigmoid(W2 @ h + b2)
                ca[p] = singles.tile([P, 2, 4], fp32, tag=f"ca{p}", name=f"ca{p}")
                for s in (0, 1):
                    for i in range(4):
                        cp = psB.tile([P, 1], fp32, tag="tiny")
                        nc.tensor.matmul(
                            cp, lhsT=w2v[:, p, i // 2, s, :],
                            rhs=h_m[p][:, i:i + 1], start=True, stop=True)
                        nc.scalar.activation(
                            out=ca[p][:, s, i:i + 1], in_=cp, func=Act.Sigmoid,
                            bias=b2v[:, p, s, i:i + 1], scale=1.0)
                weff16[p] = singles.tile([P, 2, 4], fp16, tag=f"we{p}",
                                         name=f"we{p}")
                nc.vector.tensor_tensor(out=weff16[p], in0=ca[p],
                                        in1=sawv[:, p], op=Alu.mult)

            def gate_p(p):
                zp = []
                zsum = singles.tile([P, 4, 2], fp32, tag=f"zs{p}", name=f"zs{p}")
                for i in range(4):
                    g = p + 2 * i
                    sps = psA.tile([P, PX], fp32, tag="sa")
                    for s in (0, 1):
                        # rank-1 "broadcast" matmul: every output partition
                        # computes the same spatial sum row
                        nc.tensor.matmul(
                            sps,
                            lhsT=weff16[p][:, s, i:i + 1].to_broadcast((P, P)),
                            rhs=a16[p][:, s, :],
                            start=(s == 0), stop=(s == 1))
                    sarep = work.tile([P, PX], fp32, tag="sarep")
                    nc.scalar.activation(
                        out=sarep, in_=sps, func=Act.Sigmoid,
                        bias=sabv[:, g:g + 1], scale=1.0)
                    z_pair = zpool.tile([P, 2, PX], fp32, tag="z")
                    t_pair = work.tile([P, 2, PX], fp32, tag="t")
                    nc.vector.tensor_tensor(
                        out=t_pair, in0=a_sb[p],
                        in1=sarep[:, None, :].to_broadcast((P, 2, PX)),
                        op=Alu.mult)
                    for s in (0, 1):
                        nc.scalar.activation(
                            out=z_pair[:, s, :], in_=t_pair[:, s, :],
                            func=Act.Sigmoid, scale=ca[p][:, s, i:i + 1],
                            accum_out=zsum[:, i, s:s + 1])
                    zp.append((i, z_pair))
                # group means (negated, scaled) replicated on all partitions
                zr = singles.tile([P, 4, 2], fp32, tag=f"zr{p}", name=f"zr{p}")
                nc.gpsimd.partition_all_reduce(
                    zr, zsum, channels=P, reduce_op=bass_isa.ReduceOp.add)
                nm = singles.tile([P, 4], fp32, tag=f"nm{p}", name=f"nm{p}")
                nc.vector.tensor_reduce(nm, zr, axis=mybir.AxisListType.X,
                                        op=Alu.add)
                nc.vector.tensor_scalar_mul(nm, nm, -1.0 / 65536.0)
                # mask + output, pair-batched [128, 2, 256]
                for i, z_pair in zp:
                    g = p + 2 * i
                    sg = work.tile([P, 2, PX], fp32, tag="sg")
                    nc.scalar.activation(
                        out=sg, in_=z_pair, func=Act.Sign,
                        bias=nm[:, i:i + 1], scale=1.0)
                    m1 = work.tile([P, 2, PX], fp32, tag="m1")
                    nc.vector.tensor_tensor(out=m1, in0=z_pair, in1=sg,
                                            op=Alu.max)
                    if i % 2 == 0:
                        nc.scalar.add(m1, m1, 1.0)
                    else:
                        nc.vector.tensor_scalar_add(m1, m1, 1.0)
                    ot = work.tile([P, 2, PX], fp32, tag="ot")
                    nc.vector.tensor_tensor(out=ot, in0=a_sb[p], in1=m1,
                                            op=Alu.mult)
                    eng = nc.sync if i % 2 == 0 else nc.gpsimd
                    eng.dma_start(
                        out=out_d.rearrange("(i x) P f -> P i x f", i=4)
                        [:, i, 2 * p:2 * p + 2, :],
                        in_=ot)

            conv_m(0)
            conv_m(1)
            fc_chain(0)
            gate_p(0)
            conv_m(2)
            conv_m(3)
            fc_chain(1)
            gate_p(1)

    nc.finalize()
    return nc


def _prep_core_inputs(x4b, w, s16, s32):
    x = np.ascontiguousarray(
        x4b.reshape(KT, P, PX).transpose(1, 0, 2)).astype(np.float16)
    return {"x": x, "w": w, "s16": s16, "s32": s32}


def _prep_params(cov4_w, cov4_b, fc1_w, fc1_b, fc2_w, fc2_b, sa_w, sa_b):
    f32 = np.float32
    w2d = np.asarray(cov4_w, f32).reshape(512, 2048)
    wr = w2d.reshape(MT, P, KT, P)                            # [m, mc, kt, part]
    w_arr = np.ascontiguousarray(wr.transpose(0, 3, 2, 1)).astype(np.float16)

    fc1_w = np.asarray(fc1_w, f32)
    fc1_b = np.asarray(fc1_b, f32)
    fc2_w = np.asarray(fc2_w, f32)
    fc2_b = np.asarray(fc2_b, f32)
    sa_w = np.asarray(sa_w, f32)
    sa_b = np.asarray(sa_b, f32)

    w1 = np.zeros((P, 2, 2, 256), f32)
    w2 = np.zeros((P, 2, 2, 2, P), f32)
    b1 = np.zeros((P, 2, 2), f32)
    b2 = np.zeros((P, 2, 2, 4), f32)
    saw = np.zeros((P, 2, 2, 4), f32)
    for p in range(2):
        W1s = np.concatenate([fc1_w[p + 2 * i] for i in range(4)], axis=0)
        b1s = np.concatenate([fc1_b[p + 2 * i] for i in range(4)], axis=0)
        for kt in range(2):
            w1[:, p, kt, :] = W1s[:, kt * P:(kt + 1) * P].T
        b1[:, p, 0] = b1s[:P]
        b1[:, p, 1] = b1s[P:]
        for pr in range(2):
            for half in range(2):
                i = 2 * pr + half
                g = p + 2 * i
                for s in range(2):
                    w2[64 * half:64 * half + 64, p, pr, s, :] = \
                        fc2_w[g][s * P:(s + 1) * P, :].T
        for i in range(4):
            g = p + 2 * i
            for s in range(2):
                b2[:, p, s, i] = fc2_b[g, s * P:(s + 1) * P]
                saw[:, p, s, i] = sa_w[g, s * P:(s + 1) * P]

    s16 = np.empty((P, _NSM16), np.float16)
    s16[:, 0:1024] = w1.reshape(P, 1024).astype(np.float16)
    s16[:, 1024:2048] = w2.reshape(P, 1024).astype(np.float16)

    s32 = np.zeros((P, _NSM32), f32)
    s32[:, _B_OFF:_B_OFF + 4] = np.asarray(cov4_b, f32).reshape(4, P).T
    s32[:, _B1_OFF:_B1_OFF + 4] = b1.reshape(P, 4)
    s32[:, _B2_OFF:_B2_OFF + 16] = b2.reshape(P, 16)
    s32[:, _SAW_OFF:_SAW_OFF + 16] = saw.reshape(P, 16)
    s32[:, _SAB_OFF:_SAB_OFF + 8] = np.broadcast_to(sa_b, (P, 8))
    return w_arr, s16, s32


def kernel(**inputs):
    from concourse.bass_utils import run_bass_kernel_spmd

    if "nc" not in _CACHE:
        _CACHE["nc"] = _build_program()
    nc = _CACHE["nc"]

    x4 = np.asarray(inputs["x4"], np.float32)
    B = x4.shape[0]
    w_arr, s16, s32 = _prep_params(
        inputs["cov4_w"], inputs["cov4_b"],
        inputs["gce_fc1_w"], inputs["gce_fc1_b"],
        inputs["gce_fc2_w"], inputs["gce_fc2_b"],
        inputs["gce_sa_w"], inputs["gce_sa_b"])

    in_maps = [
        _prep_core_inputs(x4[b].reshape(2048, PX), w_arr, s16, s32)
        for b in range(B)
    ]
    res = run_bass_kernel_spmd(nc, in_maps, list(range(N_CORES)))
    _CACHE["last_results"] = res

    out = np.empty((B, 2048, 16, 16), np.float32)
    for b in range(B):
        out[b] = res.results[b]["out"].reshape(2048, 16, 16)
    return out
